# revision 24
# baseline (speedup 1.0000x reference)
"""GAE (generalized advantage estimation) Trainium2 kernel.

Problem: nn_CustomGAE — B=512, T=2048, D=64.
  value = obs @ W + b ; next_value = next_obs @ W + b
  td0 = reward + gamma*nd*next_value - value ; coef = gamma*lambda*nd
  A_t = td0_t + coef_t * A_{t+1}  (reverse scan over T, independent per trajectory)
  returns (advantage, value_target = advantage + value)

Sharding: pure data parallel over B across 8 cores (64 trajectories/core).

Per-core layout: the host pre-swizzles each 64-trajectory shard to
(half, batch)-major, so SBUF partition p = h*64 + b holds timesteps
t in [h*1024, (h+1)*1024) at a uniform DRAM stride — every streamed chunk is
one 128-partition dma_start with contiguous per-partition runs. The host also
casts obs/next_obs to bf16 (the 2e-2 tolerance leaves ~6x margin), which
halves the stream DMA bytes and unlocks the DVE packed-2-byte fast mode.

Pipeline (HW-measured design points):
  - obs streams on the SP HWDGE ring, next_obs on the Activation ring;
    output batches ride the Activation ring between input chunks.
  - Both streams of a chunk land in ONE tile so the value-head multiply and
    the halving tree folds are single DVE instructions (<=16384 elements —
    larger merged ops regress on HW). TensorReduce has no packed fast mode
    and Pool is ~3x slower per element than DVE for big elementwise ops, so
    D=64 is TT-folded down to 4 in the 2x mode and one strided-output
    TensorReduce writes v/nv into the two halves of the vn tile. Everything
    stays on DVE: offloading the small td0 ops to Pool measured SLOWER
    (cross-engine sync on the scan path beats the compute saved).
  - Chunks stream in REVERSE time order, ramped sizes at both ends (small
    first chunks start DVE sooner, small last chunks shrink the tail).
    Each chunk's td0 is computed on arrival and the late-time half
    (partitions 64..127) is scanned immediately, chaining `initial` across
    chunks; its outputs stream out per flush batch. The early-time half's
    td0 persists, and after the cross-half carry hops partitions via one
    tiny SBUF->SBUF DMA, a single full-length scan (~1us: the scan costs
    free-dim cycles only, all partitions in parallel) plus a two-piece
    finalize/writeback forms the only serial tail after the last byte.
"""

import sys

sys.path.insert(0, "/opt/trn_rl_repo")

from contextlib import ExitStack

import numpy as np

import concourse.bacc as bacc
import concourse.mybir as mybir
import concourse.tile as tile
import ml_dtypes
from concourse.bass_utils import run_bass_kernel_spmd

GAMMA = 0.99
LMBDA = 0.95

B, T, D = 512, 2048, 64
NCORES = 8
BL = B // NCORES  # 64 trajectories per core
H = 2  # trajectory halves stacked on partitions -> 128 partitions
P = H * BL  # 128
F32 = mybir.dt.float32
BF16 = mybir.dt.bfloat16
U8 = mybir.dt.uint8

# obs/next_obs stream dtype: host casts to bf16 (value head is far inside the
# 2e-2 tolerance) which halves stream DMA and enables DVE packed-dtype modes.
IN_DT = "bf16"
IN_NP = ml_dtypes.bfloat16 if IN_DT == "bf16" else np.float32

# Results of the last hardware run, for test harnesses.
LAST_RESULTS = None


def _rsl(off, sz):
    """Reversed free-dim slice covering columns [off, off+sz)."""
    return slice(off + sz - 1, (off - 1) if off > 0 else None, -1)


def _build_iter2(
    nc, opool, qpool, dpool, w_t, b_t,
    obs_d, nobs_d, rw_d, dn_d, adv_d, tgt_d, tp, sizes,
    sdt, red_mode="tree", fold_w=4, nv_on="dve", small_on="dve", out_batch=192,
):
    """One full pass, reverse-streamed with incremental blocked scan."""
    mult = mybir.AluOpType.mult
    add = mybir.AluOpType.add
    sub = mybir.AluOpType.subtract
    bypass = mybir.AluOpType.bypass
    X = mybir.AxisListType.X
    oeng = nc.scalar  # output DMAs ride the Activation ring between inputs

    rw_t = dpool.tile([P, tp], F32)
    dn_t = dpool.tile([P, tp], U8)
    ndf = dpool.tile([P, tp], F32)
    g = dpool.tile([P, tp], F32)  # gamma * nd
    coef = dpool.tile([P, tp], F32)  # gamma * lambda * nd
    rw2 = rw_t  # rw + b*(g-1), in place over rw

    vdt = sdt if red_mode == "direct" else F32
    # vn holds both value heads: v = vn[:, :tp], nv = vn[:, tp:], so one
    # strided-output TensorReduce per chunk covers both streams.
    vn = dpool.tile([P, 2 * tp], vdt)
    v_raw = vn[:, :tp]
    adv = dpool.tile([P, tp], F32)
    td0 = dpool.tile([P, tp], F32)
    tgt = dpool.tile([P, tp], F32)
    bnd = dpool.tile([BL, 1], F32)

    hi = slice(BL, 2 * BL)
    lo = slice(0, BL)
    bulk = max(sizes)
    wb = {
        sz: w_t[:].unsqueeze(1).broadcast_to([P, 2 * sz, D])
        for sz in sorted(set(sizes))
    }

    off = tp
    prev_off = None  # column of the previously scanned chunk's first element
    out_hwm = tp  # columns [out_hwm, tp) already written back (hi half)
    for j, sz in enumerate(sizes):
        off -= sz
        cs = slice(off, off + sz)
        fs = slice(off * D, (off + sz) * D)
        # obs chunk lands in the first half of the tile, next_obs (other DMA
        # ring) in the second, so the value-head mult and the halving folds
        # cover both streams in single instructions.
        ot = opool.tile([P, 2 * bulk * D], sdt)
        nc.sync.dma_start(ot[:, : sz * D], obs_d.ap()[:, fs])
        nc.scalar.dma_start(ot[:, sz * D : 2 * sz * D], nobs_d.ap()[:, fs])
        x3 = ot[:, : 2 * sz * D].rearrange("p (t d) -> p t d", d=D)
        if j == 0:
            # rw/dn ride the rings BEHIND the first input chunks; the decay
            # factors (Pool, idle otherwise) are ready before chunk 0's
            # epilogue needs them. rw2 = rw + b*(g-1) folds the value-head
            # bias out of the per-chunk path: td0 = rw + g*(nv+b) - (v+b)
            # = (g*nv - v) + rw2, all plain tensor_tensor. (g-1 overwrites
            # ndf, rw2 overwrites rw in place to save SBUF.)
            nc.sync.dma_start(rw_t[:], rw_d.ap())
            nc.sync.dma_start(dn_t[:], dn_d.ap())
            nc.gpsimd.tensor_copy(ndf[:], dn_t[:])  # u8 -> f32
            nc.gpsimd.tensor_scalar(g[:], ndf[:], -GAMMA, GAMMA, mult, add)
            nc.gpsimd.tensor_scalar(
                coef[:], ndf[:], -GAMMA * LMBDA, GAMMA * LMBDA, mult, add
            )
            nc.gpsimd.tensor_scalar(ndf[:], ndf[:], -GAMMA, GAMMA - 1.0, mult, add)

        # value head: in-place mult, then fold D down in the 2-byte packed
        # fast mode (TensorReduce has no packed mode; free-axis reduces are
        # DVE-only and Pool is ~3x slower per element on this HW). One TR
        # writes v into vn[:, off:off+sz] and nv into vn[:, tp+off:...] via
        # a strided [p, 2, sz] output view.
        vno = vn[:].rearrange("p (h t) -> p h t", h=2)[:, :, off : off + sz]
        nvs = vn[:, tp + off : tp + off + sz]
        nc.vector.tensor_tensor(out=x3, in0=x3, in1=wb[sz], op=mult)
        if red_mode == "direct":
            nc.vector.tensor_reduce(out=vno, in_=x3, axis=X, op=add)
        else:
            w2 = D // 2
            while w2 >= fold_w:
                nc.vector.tensor_tensor(
                    out=x3[:, :, :w2], in0=x3[:, :, :w2],
                    in1=x3[:, :, w2 : 2 * w2], op=add,
                )
                w2 //= 2
            x4 = x3[:, :, : 2 * w2].rearrange("p (h t) d -> p h t d", h=2)
            nc.vector.tensor_reduce(out=vno, in_=x4, axis=X, op=add)
        seng = nc.gpsimd if small_on == "pool" else nc.vector
        if j == 0:
            nc.vector.scalar_tensor_tensor(
                out=rw2[:], in0=ndf[:], scalar=b_t[:, 0:1], in1=rw_t[:],
                op0=mult, op1=add,
            )

        # td0 = g*nv - v + rw2, all Pool-eligible tensor_tensor ops,
        # written into a persistent tile (the early-time half is scanned in
        # one shot at the tail)
        seng.tensor_tensor(out=td0[:, cs], in0=g[:, cs], in1=nvs, op=mult)
        t1 = qpool.tile([P, bulk], F32)
        seng.tensor_tensor(out=t1[:, :sz], in0=rw2[:, cs], in1=v_raw[:, cs], op=sub)
        seng.tensor_tensor(out=td0[:, cs], in0=td0[:, cs], in1=t1[:, :sz], op=add)

        # incremental reverse scan of the late-time half, chained via
        # `initial`; the early half needs the cross-half carry, so it is
        # scanned once at the tail (the scan costs only free-dim cycles --
        # all partitions run in parallel)
        rs = _rsl(off, sz)
        a_init = 0.0 if prev_off is None else adv[hi, prev_off : prev_off + 1]
        nc.vector.tensor_tensor_scan(
            out=adv[hi, rs], data0=coef[hi, rs], data1=td0[hi, rs],
            initial=a_init, op0=mult, op1=add,
        )
        prev_off = off

        # late-time half is exact: finalize + stream out per flush batch
        if out_hwm - off >= out_batch or j == len(sizes) - 1:
            ob = slice(off, out_hwm)
            nc.vector.scalar_tensor_tensor(
                out=tgt[hi, ob], in0=adv[hi, ob], scalar=b_t[BL:, 0:1],
                in1=v_raw[hi, ob], op0=add, op1=add,
            )
            oeng.dma_start(adv_d.ap()[hi, ob], adv[hi, ob])
            oeng.dma_start(tgt_d.ap()[hi, ob], tgt[hi, ob])
            out_hwm = off

    # tail: carry the boundary value across halves, then scan the whole
    # early-time half in one shot and finalize it in two column halves so
    # the first writeback DMA overlaps the second half's compute.
    nc.sync.dma_start(bnd[:], adv[hi, 0:1])
    nc.vector.tensor_tensor_scan(
        out=adv[lo, ::-1], data0=coef[lo, ::-1], data1=td0[lo, ::-1],
        initial=bnd[:, 0:1], op0=mult, op1=add,
    )
    half = tp // 2
    for c0, c1 in ((0, half), (half, tp)):
        cs = slice(c0, c1)
        nc.vector.scalar_tensor_tensor(
            out=tgt[lo, cs], in0=adv[lo, cs], scalar=b_t[:BL, 0:1],
            in1=v_raw[lo, cs], op0=add, op1=add,
        )
        nc.sync.dma_start(adv_d.ap()[lo, cs], adv[lo, cs])
        nc.scalar.dma_start(tgt_d.ap()[lo, cs], tgt[lo, cs])


def build_program(
    t_total=T, bulk=128, head_sizes=(32, 96), tail_sizes=(64, 32, 16, 8, 8),
    repeat=1,
    in_dt=None, red_mode="tree", fold_w=4, nv_on="dve", small_on="dve",
    out_batch=192, bufs=3, dbl=2, bench_internal=False,
):
    """Build the per-core Bass program (all 8 cores run it SPMD on their own
    shard). DRAM tensor layouts are (half, batch)-major as produced by
    shard_inputs. repeat>1 re-runs the whole pipeline inside one NEFF
    (test.py uses the delta vs repeat=1 to measure per-iteration HW time).
    bench_internal makes obs/next_obs Internal DRAM (not shipped per call;
    garbage values) so benchmark calls are cheap — timing-only builds."""
    tp = t_total // H  # timesteps per partition
    rest = tp - sum(tail_sizes) - sum(head_sizes)
    if rest < 0 or rest % bulk:  # small builds: drop the startup ramp
        head_sizes = ()
        rest = tp - sum(tail_sizes)
    assert rest >= 0 and rest % bulk == 0
    sizes = list(head_sizes) + [bulk] * (rest // bulk) + list(tail_sizes)
    assert sum(sizes) == tp

    nc = bacc.Bacc(
        "TRN2", target_bir_lowering=False, debug=False, enable_asserts=False
    )

    if in_dt is None:
        in_dt = IN_DT
    sdt = BF16 if in_dt == "bf16" else F32
    big_kind = "Internal" if bench_internal else "ExternalInput"
    obs_d = nc.dram_tensor("obs", [P, tp * D], sdt, kind=big_kind)
    nobs_d = nc.dram_tensor("nobs", [P, tp * D], sdt, kind=big_kind)
    rw_d = nc.dram_tensor("rw", [P, tp], F32, kind="ExternalInput")
    dn_d = nc.dram_tensor("dn", [P, tp], U8, kind="ExternalInput")
    w_d = nc.dram_tensor("w", [D], sdt, kind="ExternalInput")
    b_d = nc.dram_tensor("b", [1], F32, kind="ExternalInput")
    adv_d = nc.dram_tensor("adv", [P, tp], F32, kind="ExternalOutput")
    tgt_d = nc.dram_tensor("tgt", [P, tp], F32, kind="ExternalOutput")

    with tile.TileContext(nc) as tc, ExitStack() as ctx:
        cpool = ctx.enter_context(tc.tile_pool(name="const", bufs=1))
        opool = ctx.enter_context(tc.tile_pool(name="obs", bufs=bufs))
        qpool = ctx.enter_context(tc.tile_pool(name="chunk", bufs=3))
        dpool = ctx.enter_context(tc.tile_pool(name="iter", bufs=dbl))

        # Value-head weights replicated to every partition.
        w_t = cpool.tile([P, D], sdt)
        nc.sync.dma_start(w_t[:], w_d.ap().unsqueeze(0).broadcast_to([P, D]))
        b_t = cpool.tile([P, 1], F32)
        nc.sync.dma_start(b_t[:], b_d.ap().unsqueeze(0).broadcast_to([P, 1]))

        with nc.allow_low_precision("bf16 value head; tolerance is 2e-2"):
            for _rep in range(repeat):
                _build_iter2(
                    nc, opool, qpool, dpool, w_t, b_t,
                    obs_d, nobs_d, rw_d, dn_d, adv_d, tgt_d, tp, sizes,
                    sdt, red_mode=red_mode, fold_w=fold_w, nv_on=nv_on,
                    small_on=small_on, out_batch=out_batch,
                )

    # Runs the bacc pipeline (register allocation etc.) — required before
    # serializing for the walrus compiler.
    nc.finalize()
    return nc


_NC_CACHE = None


def _get_nc():
    global _NC_CACHE
    if _NC_CACHE is None:
        _NC_CACHE = build_program()
    return _NC_CACHE


def _hmajor(x, tp_cols):
    """[BL, H*tp_cols] row-major -> [H*BL, tp_cols] with row p = h*BL + b."""
    return np.ascontiguousarray(
        x.reshape(BL, H, tp_cols).transpose(1, 0, 2).reshape(H * BL, tp_cols)
    )


def _unhmajor(y):
    """Inverse of _hmajor for outputs: [H*BL, tp] -> [BL, H*tp]."""
    tp = y.shape[1]
    return y.reshape(H, BL, tp).transpose(1, 0, 2).reshape(BL, H * tp)


def shard_inputs(obs, next_obs, reward, done, W, b):
    """Split full inputs into the 8 per-core input maps ((h,b)-major)."""
    obs = np.asarray(obs, dtype=IN_NP).reshape(B, T * D)
    nobs = np.asarray(next_obs, dtype=IN_NP).reshape(B, T * D)
    rw = np.asarray(reward, dtype=np.float32).reshape(B, T)
    dn = np.asarray(done).astype(np.uint8, copy=False).reshape(B, T)
    w_np = np.ascontiguousarray(np.asarray(W, dtype=IN_NP)).reshape(D)
    b_np = np.ascontiguousarray(np.asarray(b, dtype=np.float32)).reshape(1)

    tpd = (T // H) * D
    tp = T // H
    in_maps = []
    for i in range(NCORES):
        sl = slice(i * BL, (i + 1) * BL)
        in_maps.append(
            {
                "obs": _hmajor(obs[sl], tpd),
                "nobs": _hmajor(nobs[sl], tpd),
                "rw": _hmajor(rw[sl], tp),
                "dn": _hmajor(dn[sl], tp),
                "w": w_np,
                "b": b_np,
            }
        )
    return in_maps


def gather_outputs(results):
    advantage = np.concatenate(
        [_unhmajor(r["adv"]) for r in results], axis=0
    ).reshape(B, T, 1)
    value_target = np.concatenate(
        [_unhmajor(r["tgt"]) for r in results], axis=0
    ).reshape(B, T, 1)
    return advantage, value_target


def kernel(obs, next_obs, reward, done, W, b):
    global LAST_RESULTS
    nc = _get_nc()
    in_maps = shard_inputs(obs, next_obs, reward, done, W, b)
    res = run_bass_kernel_spmd(nc, in_maps, core_ids=list(range(NCORES)))
    LAST_RESULTS = res
    return gather_outputs(res.results)
